# revision 1
# baseline (speedup 1.0000x reference)
"""DirGCNConv (weighted gather + segment_sum + linear) on 8 Trainium2 NeuronCores.

Computation (reference):
    dst, src = edge_index
    agg[d] = sum_{e: dst_e == d} edge_weight[e] * x[src_e]     # [N, D]
    out = agg @ W.T + b

Strategy (dst-sharded, no collectives):
  - Host: shard edges by dst node range (N/8 per core). Within a core, sort
    edges by dst, pack consecutive dst nodes into tiles (<=256 nodes and
    <=8192 edges per tile, padded to exactly 8192 edge slots with w=0
    duplicates via a proportional stretch of the src-sorted edge list).
    x is cast to bf16 on host (tolerance gate is 2e-2).
  - Device, per tile (64 chunks of 128 edges):
      * per chunk: one vector-indirect DMA (InstDMACopy on qPoolDynamic,
        one int32 row index per partition) gathers 128 rows of 256B from
        the bf16 node table: G[p, :] = xt[idx[p]].
        (The much faster Ant dma_gather ucode crashes this runtime —
        NRT_EXEC_UNIT_UNRECOVERABLE — so the standard INDIRECT1D path is
        used; it costs ~1.4us per chunk on the Pool engine and dominates.)
      * one DVE tensor_scalar builds the weighted one-hot
        S[e, dstloc] = (iota == dstloc_e) * w_e  [128x256 bf16],
        then TensorE matmul accumulates PSUM agg.T[din, dst256] += G.T @ S
        (lhsT = gathered chunk [128 edges x 128 din]).
      * tail: PSUM -> SBUF (ACT), out.T = W @ agg.T (fp32 matmul with
        stationary W.T), + bias via ACT per-partition bias, DMA out.
  - Host: reassemble out from per-core out.T tiles (pure index mapping).
"""

import numpy as np
import ml_dtypes

bf16 = ml_dtypes.bfloat16

# problem constants (hardcoded per harness contract)
N_NODES = 100000
N_EDGES = 3200000
D = 128
NCORES = 8

# design constants
TILE_W = 256        # dst columns per tile (S width, psum free dim)
TILE_E = 8192       # edges per tile (padded)
CHUNK = 128         # edges per matmul (contraction dim)


def _prep_core(dst_l, src_c, w_c, n_local):
    """Prepare one core's tile structure.

    dst_l: local dst ids, sorted ascending. src_c, w_c aligned with dst_l.
    Returns dict of per-core arrays (without cross-core T padding) plus the
    node->column map.
    """
    deg = np.bincount(dst_l, minlength=n_local)
    cum = np.concatenate([[0], np.cumsum(deg)])

    # greedy tiling: consecutive nodes, <= TILE_W nodes, <= TILE_E edges
    tiles = []
    a = 0
    while a < n_local:
        b = int(np.searchsorted(cum, cum[a] + TILE_E, side="right")) - 1
        b = min(b, a + TILE_W, n_local)
        assert b > a, f"node {a} has degree {deg[a]} > TILE_E"
        tiles.append((a, b))
        a = b
    T = len(tiles)

    src_buf = np.zeros((T, TILE_E), np.int64)
    w_buf = np.zeros((T, TILE_E), np.float32)
    dl_buf = np.zeros((T, TILE_E), np.int64)

    # bonus (paired src+1) bookkeeping: per tile, which edges ride the second
    # gathered row of their anchor's window instead of a main slot.
    bonus_src = [None] * T   # anchor srcs actually used (for matching audit)
    bonus_dl = np.zeros((T, TILE_E), np.int64)
    bonus_w = np.zeros((T, TILE_E), np.float32)
    n_matched = np.zeros(T, np.int64)
    main_edges = [None] * T

    for t, (a, b) in enumerate(tiles):
        ea, eb = int(cum[a]), int(cum[b])
        cnt = eb - ea
        assert 0 < cnt <= TILE_E
        s_sl = src_c[ea:eb]
        o = np.argsort(s_sl, kind="stable")
        ss = s_sl[o]
        ww = w_c[ea:eb][o]
        dd = dst_l[ea:eb][o] - a
        # greedy src+1 pairing: edge j is a bonus for anchor i if
        # ss[j] == ss[i] + 1; each anchor carries at most one bonus.
        # ss sorted: use searchsorted runs.
        lo = np.searchsorted(ss, ss + 1, side="left")
        hi = np.searchsorted(ss, ss + 1, side="right")
        is_bonus = np.zeros(cnt, bool)
        anchor_of = np.full(cnt, -1, np.int64)  # bonus j -> anchor i
        taken = np.zeros(cnt, bool)
        for i in range(cnt):
            if is_bonus[i]:
                continue
            for j in range(lo[i], hi[i]):
                if not taken[j] and not is_bonus[j] and anchor_of[j] < 0 \
                        and not is_bonus[i]:
                    if j == i:
                        continue
                    is_bonus[j] = True
                    anchor_of[j] = i
                    taken[j] = True
                    break
        n_matched[t] = int(is_bonus.sum())
        main_edges[t] = (ss, ww, dd, is_bonus, anchor_of)

    return {
        "T": T,
        "tiles": tiles,
        "cum": cum,
        "n_matched": n_matched,
        "main_edges": main_edges,
        "bonus_dl": bonus_dl,
        "bonus_w": bonus_w,
    }


def _finalize_core(prep, bonus_cap):
    """Given per-tile bonus capacities (static across cores), build streams."""
    T = prep["T"]
    nchunk_t = [(TILE_E - int(c)) // CHUNK for c in bonus_cap[:T]]
    src_buf, w_buf, dl_buf = [], [], []
    bdl_buf, bw_buf = [], []
    for t in range(T):
        ss, ww, dd, is_bonus, anchor_of = prep["main_edges"][t]
        cnt = len(ss)
        cap = int(bonus_cap[t])
        nslots = TILE_E - cap
        # choose up to cap bonus edges; demote the rest to main
        bidx = np.flatnonzero(is_bonus)[:cap]
        keep_bonus = np.zeros(cnt, bool)
        keep_bonus[bidx] = True
        midx = np.flatnonzero(~keep_bonus)
        n_main = len(midx)
        assert n_main <= nslots, (n_main, nslots)
        # proportional stretch of main edges into nslots
        pos = (np.arange(nslots, dtype=np.int64) * n_main) // nslots
        first = np.ones(nslots, bool)
        first[1:] = pos[1:] != pos[:-1]
        sel = midx[pos]
        sbuf = ss[sel]
        wbuf = np.where(first, ww[sel], 0.0)
        dbuf = np.where(first, dd[sel], 0)
        # bonus placement: first slot position of each anchor in the
        # stretched stream
        slot_of = np.full(cnt, -1, np.int64)
        slot_of[sel[first]] = np.flatnonzero(first)
        bdl = np.zeros(nslots, np.int64)
        bw = np.zeros(nslots, np.float32)
        for j in bidx:
            anc = anchor_of[j]
            p = slot_of[anc]
            assert p >= 0, "anchor lost in stretch"
            bdl[p] = dd[j]
            bw[p] = ww[j]
        src_buf.append(sbuf)
        w_buf.append(wbuf)
        dl_buf.append(dbuf)
        bdl_buf.append(bdl)
        bw_buf.append(bw)

    def cols(bufs):
        out = []
        for t, bufv in enumerate(bufs):
            nck = nchunk_t[t]
            out.append(bufv.reshape(nck, CHUNK).T)
        return np.concatenate(out, axis=1)

    return {
        "T": T,
        "tiles": prep["tiles"],
        "nchunk_t": nchunk_t,
        "idx": np.ascontiguousarray(cols(src_buf).astype(np.int32)),
        "dl": np.ascontiguousarray(cols(dl_buf).astype(np.float32)),
        "w": np.ascontiguousarray(cols(w_buf).astype(np.float32)),
        "bdl": np.ascontiguousarray(cols(bdl_buf).astype(np.float32)),
        "bw": np.ascontiguousarray(cols(bw_buf).astype(np.float32)),
    }

def _build_program(nchunk_t, pad_rows, gather_bufs=8, s_bufs=6):
    import concourse.bass as bass
    import concourse.bacc as bacc
    import concourse.mybir as mybir
    import concourse.tile as tile

    T = len(nchunk_t)
    tot = int(sum(nchunk_t))
    off = np.concatenate([[0], np.cumsum(nchunk_t)]).astype(int)

    nc = bacc.Bacc("TRN2", target_bir_lowering=False, debug=False,
                   num_devices=NCORES)

    xt_d = nc.dram_tensor("xt", [pad_rows, D], mybir.dt.bfloat16,
                          kind="ExternalInput")
    idx_d = nc.dram_tensor("idx", [128, tot], mybir.dt.int32, kind="ExternalInput")
    dl_d = nc.dram_tensor("dl", [128, tot], mybir.dt.float32, kind="ExternalInput")
    w_d = nc.dram_tensor("w", [128, tot], mybir.dt.float32, kind="ExternalInput")
    any_bonus = any(int(n) < TILE_E // CHUNK for n in nchunk_t)
    if any_bonus:
        bdl_d = nc.dram_tensor("bdl", [128, tot], mybir.dt.float32, kind="ExternalInput")
        bw_d = nc.dram_tensor("bw", [128, tot], mybir.dt.float32, kind="ExternalInput")
    iota_d = nc.dram_tensor("iota", [128, TILE_W], mybir.dt.bfloat16,
                            kind="ExternalInput")
    wt_d = nc.dram_tensor("wt", [D, D], mybir.dt.float32, kind="ExternalInput")
    b_d = nc.dram_tensor("b", [D, 1], mybir.dt.float32, kind="ExternalInput")
    out_d = nc.dram_tensor("outT", [D, T * TILE_W], mybir.dt.float32,
                           kind="ExternalOutput")

    with tile.TileContext(nc) as tc:
        with (
            tc.tile_pool(name="const", bufs=1) as constp,
            tc.tile_pool(name="meta", bufs=3) as metap,
            tc.tile_pool(name="gather", bufs=gather_bufs) as gatherp,
            tc.tile_pool(name="s", bufs=s_bufs) as sp,
            tc.tile_pool(name="agg", bufs=2) as aggp,
            tc.tile_pool(name="outp", bufs=2) as outp,
            tc.tile_pool(name="psum", bufs=2, space="PSUM") as psump,
            tc.tile_pool(name="psum2", bufs=2, space="PSUM") as psum2p,
        ):
            iota_t = constp.tile([128, TILE_W], mybir.dt.bfloat16)
            wt_t = constp.tile([D, D], mybir.dt.float32)
            b_t = constp.tile([D, 1], mybir.dt.float32)
            nc.sync.dma_start(iota_t[:], iota_d[:])
            nc.sync.dma_start(wt_t[:], wt_d[:])
            nc.sync.dma_start(b_t[:], b_d[:])

            for t in range(T):
                nck = int(nchunk_t[t])
                o0 = int(off[t])
                has_bonus = nck < TILE_E // CHUNK
                idx_t = metap.tile([128, nck], mybir.dt.int32, tag="idx")
                dl_t = metap.tile([128, nck], mybir.dt.float32, tag="dl")
                w_t = metap.tile([128, nck], mybir.dt.float32, tag="w")
                nc.sync.dma_start(idx_t[:], idx_d[:, o0:o0 + nck])
                nc.sync.dma_start(dl_t[:], dl_d[:, o0:o0 + nck])
                nc.sync.dma_start(w_t[:], w_d[:, o0:o0 + nck])
                if has_bonus:
                    bdl_t = metap.tile([128, nck], mybir.dt.float32, tag="bdl")
                    bw_t = metap.tile([128, nck], mybir.dt.float32, tag="bw")
                    nc.sync.dma_start(bdl_t[:], bdl_d[:, o0:o0 + nck])
                    nc.sync.dma_start(bw_t[:], bw_d[:, o0:o0 + nck])

                psum_t = psump.tile([D, TILE_W], mybir.dt.float32, tag="p1")
                for c in range(nck):
                    gshape = [128, 2, D] if has_bonus else [128, D]
                    g_t = gatherp.tile(gshape, mybir.dt.bfloat16, tag="g")
                    nc.gpsimd.indirect_dma_start(
                        out=g_t[:],
                        out_offset=None,
                        in_=xt_d[:],
                        in_offset=bass.IndirectOffsetOnAxis(
                            ap=idx_t[:, c:c + 1], axis=0),
                    )
                    s_t = sp.tile([128, TILE_W], mybir.dt.bfloat16, tag="s")
                    nc.vector.tensor_scalar(
                        s_t[:], iota_t[:], dl_t[:, c:c + 1], w_t[:, c:c + 1],
                        mybir.AluOpType.is_equal, mybir.AluOpType.mult,
                    )
                    main_lhsT = g_t[:, 0, :] if has_bonus else g_t[:]
                    nc.tensor.matmul(
                        psum_t[:], main_lhsT, s_t[:],
                        start=(c == 0),
                        stop=(c == nck - 1 and not has_bonus),
                    )
                    if has_bonus:
                        sb_t = sp.tile([128, TILE_W], mybir.dt.bfloat16, tag="sb")
                        nc.vector.tensor_scalar(
                            sb_t[:], iota_t[:], bdl_t[:, c:c + 1], bw_t[:, c:c + 1],
                            mybir.AluOpType.is_equal, mybir.AluOpType.mult,
                        )
                        nc.tensor.matmul(
                            psum_t[:], g_t[:, 1, :], sb_t[:],
                            start=False,
                            stop=(c == nck - 1),
                        )

                aggT_t = aggp.tile([D, TILE_W], mybir.dt.float32, tag="agg")
                nc.scalar.copy(aggT_t[:], psum_t[:])

                psum2_t = psum2p.tile([D, TILE_W], mybir.dt.float32, tag="p2")
                nc.tensor.matmul(psum2_t[:], wt_t[:], aggT_t[:],
                                 start=True, stop=True)

                out_t = outp.tile([D, TILE_W], mybir.dt.float32, tag="o")
                nc.scalar.activation(
                    out_t[:], psum2_t[:],
                    mybir.ActivationFunctionType.Identity,
                    bias=b_t[:, 0:1], scale=1.0,
                )
                nc.sync.dma_start(out_d[:, t * TILE_W:(t + 1) * TILE_W], out_t[:])

    nc.compile()
    return nc


def _host_prep(x, edge_index, edge_weight, n_nodes, ncores):
    """Full host-side preparation. Returns (finalized per-core preps, ...)."""
    n_local = n_nodes // ncores
    dst = np.asarray(edge_index[0], dtype=np.int64)
    src = np.asarray(edge_index[1], dtype=np.int64)
    w = np.asarray(edge_weight, dtype=np.float32)

    order = np.argsort(dst, kind="stable")
    dst_s, src_s, w_s = dst[order], src[order], w[order]
    bounds = np.searchsorted(dst_s, np.arange(ncores + 1) * n_local)

    raws = []
    for c in range(ncores):
        lo, hi = int(bounds[c]), int(bounds[c + 1])
        raws.append(_prep_core(dst_s[lo:hi] - c * n_local,
                               src_s[lo:hi], w_s[lo:hi], n_local))
    T_glob = max(p["T"] for p in raws)

    # static per-tile bonus capacity = min over cores owning that tile,
    # floored to a multiple of CHUNK (so chunk counts stay uniform)
    bonus_cap = np.zeros(T_glob, np.int64)
    import os as _os
    _nb = _os.environ.get("KERNEL_BONUS") != "1"  # bonus scheme off by default: slower+wrong on HW
    for t in range(T_glob):
        if _nb:
            continue
        m = min((int(p["n_matched"][t]) for p in raws if t < p["T"]), default=0)
        bonus_cap[t] = (m // CHUNK) * CHUNK
    nchunk_t = [(TILE_E - int(c)) // CHUNK for c in bonus_cap]

    preps = []
    for p in raws:
        f = _finalize_core(p, bonus_cap)
        # pad to T_glob tiles
        if f["T"] < T_glob:
            extra = int(sum(nchunk_t[f["T"]:]))
            for k2, dt2 in (("idx", np.int32), ("dl", np.float32),
                            ("w", np.float32), ("bdl", np.float32),
                            ("bw", np.float32)):
                f[k2] = np.concatenate(
                    [f[k2], np.zeros((CHUNK, extra), dt2)], axis=1)
        preps.append(f)

    pad_rows = n_nodes + 1  # +1: bonus row idx+1 of the last node stays in-bounds
    xt16 = np.zeros((pad_rows, D), bf16)
    xt16[:n_nodes] = np.asarray(x, np.float32).astype(bf16)

    iota = np.broadcast_to(
        np.arange(TILE_W, dtype=np.float32), (128, TILE_W)).astype(bf16)
    iota = np.ascontiguousarray(iota)

    return preps, nchunk_t, pad_rows, xt16, iota


def _assemble_output(results, preps, W, b, n_nodes, ncores):
    n_local = n_nodes // ncores
    out = np.empty((n_nodes, D), np.float32)
    for c in range(ncores):
        outT = results[c]["outT"]  # [D, T*TILE_W]
        cols = []
        nodes = []
        for t, (a, bb) in enumerate(preps[c]["tiles"]):
            cols.append(t * TILE_W + np.arange(bb - a))
            nodes.append(np.arange(a, bb))
        cols = np.concatenate(cols)
        nodes = np.concatenate(nodes) + c * n_local
        out[nodes, :] = outT[:, cols].T
    return out


LAST_RES = None


def kernel(x, edge_index, edge_weight, W, b):
    import os
    from concourse.bass_utils import run_bass_kernel_spmd

    preps, nchunk_t, pad_rows, xt16, iota = _host_prep(
        x, edge_index, edge_weight, N_NODES, NCORES)

    nc = _build_program(nchunk_t, pad_rows)

    WT = np.ascontiguousarray(np.asarray(W, np.float32).T)  # [din, dout]
    bcol = np.ascontiguousarray(np.asarray(b, np.float32).reshape(D, 1))

    in_maps = []
    for c in range(NCORES):
        p = preps[c]
        in_maps.append({
            "xt": xt16, "idx": p["idx"], "dl": p["dl"], "w": p["w"],
            "bdl": p["bdl"], "bw": p["bw"],
            "iota": iota, "wt": WT, "b": bcol,
        })

    res = run_bass_kernel_spmd(
        nc, in_maps, core_ids=list(range(NCORES)),
        trace=bool(int(os.environ.get("KERNEL_TRACE", "0"))),
    )
    global LAST_RES
    LAST_RES = res
    out = _assemble_output(res.results, preps, W, b, N_NODES, NCORES)
    return out


if __name__ == "__main__":
    # smoke test with random data (self-contained)
    rng = np.random.default_rng(0)
    x = rng.standard_normal((N_NODES, D)).astype(np.float32)
    ei = rng.integers(0, N_NODES, size=(2, N_EDGES)).astype(np.int64)
    ew = rng.random(N_EDGES).astype(np.float32)
    W = (rng.standard_normal((D, D)) / np.sqrt(D)).astype(np.float32)
    b = (rng.standard_normal(D) * 0.01).astype(np.float32)
    out = kernel(x, ei, ew, W, b)
    print("out", out.shape, out.dtype)



# revision 4
# speedup vs baseline: 1.2176x; 1.2176x over previous
"""DirGCNConv (weighted gather + segment_sum + linear) on 8 Trainium2 NeuronCores.

Computation (reference):
    dst, src = edge_index
    agg[d] = sum_{e: dst_e == d} edge_weight[e] * x[src_e]     # [N, D]
    out = agg @ W.T + b

Strategy (dst-sharded, no collectives):
  - Host: shard edges by dst node range (N/8 = 12500 nodes per core). Fixed
    dst tiles of 256 nodes (T=49 per core). Within a tile, edges are grouped
    by src bank (4 banks of 25000 rows so bank-local ids fit int16 for the
    dma_gather ucode), sorted by src for HBM row locality, and padded per
    (tile, bank) to a multiple of 128 edges; the padded count is the max
    over cores so all 8 cores share one program (SPMD).
  - Device, per group of GT=2 tiles:
      * 4 dma_gather ucode calls (InstDMAGatherAnt, mlp library) gather all
        the group's edges' source rows from the bf16 bank tables in HBM into
        an SBUF buffer G[128, nblk, 128]: stream slot i -> partition i%128,
        block i//128 (matches the 128-edge matmul chunk layout).
        One instruction per (group, bank) costs ~1us + 0.34ns/row of Pool
        time vs ~1.1us per 128 rows for the old per-chunk indirect DMA.
      * per 128-edge chunk j: DVE tensor_scalar builds the weighted one-hot
        S[e, dstloc] = (iota == dl_e) * w_e  [128x256 bf16, 4x perf mode],
        TensorE accumulates PSUM agg.T[din, dst256] += G_j.T @ S_j.
      * per tile: PSUM -> SBUF (ACT), out.T = W @ agg.T (fp32 matmul with
        stationary W.T), + bias via ACT per-partition bias, DMA out.
  - Host: out rows of core c = outT[:, :12500].T (tile t covers local nodes
    t*256..t*256+255, so column == local node id).
"""

import numpy as np
import ml_dtypes

bf16 = ml_dtypes.bfloat16

# problem constants (hardcoded per harness contract)
N_NODES = 100000
N_EDGES = 3200000
D = 128
NCORES = 8

# design constants
NLOC = N_NODES // NCORES      # 12500 dst nodes per core
TILE_W = 256                  # dst columns per tile
T_TILES = (NLOC + TILE_W - 1) // TILE_W   # 49
CHUNK = 128                   # edges per matmul chunk
NBANKS = 4
BROWS = N_NODES // NBANKS     # 25000 rows per src bank (int16-safe)
GT = 2                        # tiles per gather group


def _host_prep(x, edge_index, edge_weight):
    """Vectorized host prep. Returns per-core streams + static block counts."""
    dst = np.asarray(edge_index[0], dtype=np.int64)
    src = np.asarray(edge_index[1], dtype=np.int64)
    w = np.asarray(edge_weight, dtype=np.float32)

    order = np.argsort(dst, kind="stable")
    dst_s, src_s, w_s = dst[order], src[order], w[order]
    bounds = np.searchsorted(dst_s, np.arange(NCORES + 1) * NLOC)

    cores = []
    counts_all = np.zeros((NCORES, T_TILES, NBANKS), np.int64)
    for c in range(NCORES):
        lo, hi = int(bounds[c]), int(bounds[c + 1])
        d_l = dst_s[lo:hi] - c * NLOC
        s_l = src_s[lo:hi]
        w_l = w_s[lo:hi]
        tile = d_l // TILE_W
        bank = s_l // BROWS
        grp = tile // GT
        # sort into segment-iteration order: (group, bank, tile), src minor
        o = np.lexsort((s_l, tile, bank, grp))
        tile, bank = tile[o], bank[o]
        cores.append((d_l[o], s_l[o], w_l[o], tile, bank))
        np.add.at(counts_all[c], (tile, bank), 1)

    # static padded counts per (tile, bank): max over cores, ceil to CHUNK
    P = ((counts_all.max(axis=0) + CHUNK - 1) // CHUNK) * CHUNK  # [T, NBANKS]
    nblk = P // CHUNK

    # segment-iteration order (group, bank, tile) -> slot offsets
    n_groups = (T_TILES + GT - 1) // GT
    seg_order = []  # (t, b) in iteration order
    for g in range(n_groups):
        for b in range(NBANKS):
            for t in range(g * GT, min((g + 1) * GT, T_TILES)):
                seg_order.append((t, b))
    seg_sizes = np.array([P[t, b] for t, b in seg_order], np.int64)
    seg_starts = np.concatenate([[0], np.cumsum(seg_sizes)])
    tot_slots = int(seg_starts[-1])
    totblk = tot_slots // CHUNK
    # map (t, b) -> slot start
    seg_start_tb = np.zeros((T_TILES, NBANKS), np.int64)
    for i, (t, b) in enumerate(seg_order):
        seg_start_tb[t, b] = seg_starts[i]

    # per-group metadata for the program builder
    groups = []
    for g in range(n_groups):
        tiles_g = list(range(g * GT, min((g + 1) * GT, T_TILES)))
        g_slot0 = int(seg_start_tb[tiles_g[0], 0])
        g_blk0 = g_slot0 // CHUNK
        bank_segs = []  # (local block offset, num blocks) per bank
        for b in range(NBANKS):
            s0 = int(seg_start_tb[tiles_g[0], b])
            ln = int(sum(P[t, b] for t in tiles_g))
            bank_segs.append(((s0 - g_slot0) // CHUNK, ln // CHUNK))
        tile_blocks = []  # per tile: list of local block indices (group-rel)
        for t in tiles_g:
            blks = []
            for b in range(NBANKS):
                s0 = int(seg_start_tb[t, b])
                blks.extend(range((s0 - g_slot0) // CHUNK,
                                  (s0 - g_slot0) // CHUNK + int(nblk[t, b])))
            tile_blocks.append((t, blks))
        nblk_g = int(sum(P[t, b] for t in tiles_g for b in range(NBANKS))) // CHUNK
        groups.append({
            "blk0": g_blk0, "nblk": nblk_g,
            "bank_segs": bank_segs, "tile_blocks": tile_blocks,
        })

    # per-core streams
    per_core = []
    for c in range(NCORES):
        d_l, s_l, w_l, tile, bank = cores[c]
        ne = len(d_l)
        # edges are sorted in segment-iteration order; compute positions
        cnt_iter = np.array(
            [counts_all[c, t, b] for t, b in seg_order], np.int64)
        first = np.concatenate([[0], np.cumsum(cnt_iter)])[:-1]
        pos = (np.repeat(seg_starts[:-1], cnt_iter)
               + np.arange(ne) - np.repeat(first, cnt_iter))

        idx_stream = np.zeros(tot_slots, np.int16)
        wq_stream = np.zeros(tot_slots, np.float32)
        dl_stream = np.zeros(tot_slots, np.float32)
        idx_stream[pos] = (s_l % BROWS).astype(np.int16)
        wq_stream[pos] = w_l
        dl_stream[pos] = (d_l % TILE_W).astype(np.float32)

        idx16 = np.ascontiguousarray(
            np.tile(idx_stream.reshape(-1, 16).T, (8, 1)))   # [128, totblk*8]
        dl_cols = np.ascontiguousarray(
            dl_stream.reshape(totblk, CHUNK).T)               # [128, totblk]
        w_cols = np.ascontiguousarray(
            wq_stream.reshape(totblk, CHUNK).T)               # [128, totblk]
        per_core.append({"idx16": idx16, "dl": dl_cols, "w": w_cols})

    xb = np.asarray(x, np.float32).astype(bf16)  # [N, D]
    banks = [np.ascontiguousarray(xb[b * BROWS:(b + 1) * BROWS])
             for b in range(NBANKS)]

    iota = np.ascontiguousarray(np.broadcast_to(
        np.arange(TILE_W, dtype=np.float32), (128, TILE_W)).astype(bf16))

    return per_core, banks, iota, groups, totblk


def _build_program(groups, totblk):
    import concourse.bass as bass  # noqa: F401
    import concourse.bacc as bacc
    import concourse.mybir as mybir
    import concourse.tile as tile
    from concourse import library_config

    nc = bacc.Bacc("TRN2", target_bir_lowering=False, debug=False,
                   num_devices=NCORES)

    xb_d = [nc.dram_tensor(f"xb{b}", [BROWS, D], mybir.dt.bfloat16,
                           kind="ExternalInput") for b in range(NBANKS)]
    idx_d = nc.dram_tensor("idx16", [128, totblk * 8], mybir.dt.int16,
                           kind="ExternalInput")
    dl_d = nc.dram_tensor("dl", [128, totblk], mybir.dt.float32,
                          kind="ExternalInput")
    w_d = nc.dram_tensor("w", [128, totblk], mybir.dt.float32,
                         kind="ExternalInput")
    iota_d = nc.dram_tensor("iota", [128, TILE_W], mybir.dt.bfloat16,
                            kind="ExternalInput")
    wt_d = nc.dram_tensor("wt", [D, D], mybir.dt.float32, kind="ExternalInput")
    b_d = nc.dram_tensor("b", [D, 1], mybir.dt.float32, kind="ExternalInput")
    out_d = nc.dram_tensor("outT", [D, T_TILES * TILE_W], mybir.dt.float32,
                           kind="ExternalOutput")

    max_nblk = max(g["nblk"] for g in groups)

    with tile.TileContext(nc) as tc:
        with (
            tc.tile_pool(name="const", bufs=1) as constp,
            tc.tile_pool(name="meta", bufs=3) as metap,
            tc.tile_pool(name="gather", bufs=2) as gatherp,
            tc.tile_pool(name="s", bufs=6) as sp,
            tc.tile_pool(name="agg", bufs=2) as aggp,
            tc.tile_pool(name="outp", bufs=2) as outp,
            tc.tile_pool(name="psum", bufs=2, space="PSUM") as psump,
            tc.tile_pool(name="psum2", bufs=2, space="PSUM") as psum2p,
        ):
            nc.gpsimd.load_library(library_config.mlp)

            iota_t = constp.tile([128, TILE_W], mybir.dt.bfloat16)
            wt_t = constp.tile([D, D], mybir.dt.float32)
            b_t = constp.tile([D, 1], mybir.dt.float32)
            nc.sync.dma_start(iota_t[:], iota_d[:])
            nc.sync.dma_start(wt_t[:], wt_d[:])
            nc.sync.dma_start(b_t[:], b_d[:])

            for g in groups:
                blk0, nblk_g = g["blk0"], g["nblk"]
                idx_t = metap.tile([128, max_nblk * 8], mybir.dt.int16,
                                   tag="idx")
                dl_t = metap.tile([128, max_nblk], mybir.dt.float32, tag="dl")
                w_t = metap.tile([128, max_nblk], mybir.dt.float32, tag="w")
                nc.sync.dma_start(idx_t[:, :nblk_g * 8],
                                  idx_d[:, blk0 * 8:(blk0 + nblk_g) * 8])
                nc.sync.dma_start(dl_t[:, :nblk_g],
                                  dl_d[:, blk0:blk0 + nblk_g])
                nc.sync.dma_start(w_t[:, :nblk_g],
                                  w_d[:, blk0:blk0 + nblk_g])

                g_t = gatherp.tile([128, max_nblk, D], mybir.dt.bfloat16,
                                   tag="g")
                # SWDGE descriptor ring caps one gather at 1024 descriptors
                # (ucode-fixed), i.e. 8 blocks of 128 rows.
                MAXG = 8
                for b in range(NBANKS):
                    boff, blen = g["bank_segs"][b]
                    for off in range(0, blen, MAXG):
                        ln = min(MAXG, blen - off)
                        a = boff + off
                        nc.gpsimd.dma_gather(
                            g_t[:, a:a + ln, :],
                            xb_d[b][:],
                            idx_t[:, a * 8:(a + ln) * 8],
                            ln * CHUNK,
                            ln * CHUNK,
                            D,
                        )

                for t, blks in g["tile_blocks"]:
                    assert blks, f"tile {t} has no edge blocks"
                    psum_t = psump.tile([D, TILE_W], mybir.dt.float32,
                                        tag="p1")
                    for k, j in enumerate(blks):
                        s_t = sp.tile([128, TILE_W], mybir.dt.bfloat16,
                                      tag="s")
                        nc.vector.tensor_scalar(
                            s_t[:], iota_t[:], dl_t[:, j:j + 1],
                            w_t[:, j:j + 1],
                            mybir.AluOpType.is_equal, mybir.AluOpType.mult,
                        )
                        nc.tensor.matmul(
                            psum_t[:], g_t[:, j, :], s_t[:],
                            start=(k == 0), stop=(k == len(blks) - 1),
                        )

                    aggT_t = aggp.tile([D, TILE_W], mybir.dt.float32,
                                       tag="agg")
                    nc.scalar.copy(aggT_t[:], psum_t[:])

                    psum2_t = psum2p.tile([D, TILE_W], mybir.dt.float32,
                                          tag="p2")
                    nc.tensor.matmul(psum2_t[:], wt_t[:], aggT_t[:],
                                     start=True, stop=True)

                    out_t = outp.tile([D, TILE_W], mybir.dt.float32, tag="o")
                    nc.scalar.activation(
                        out_t[:], psum2_t[:],
                        mybir.ActivationFunctionType.Identity,
                        bias=b_t[:, 0:1], scale=1.0,
                    )
                    nc.sync.dma_start(
                        out_d[:, t * TILE_W:(t + 1) * TILE_W], out_t[:])

    nc.compile()
    return nc


LAST_RES = None


def kernel(x, edge_index, edge_weight, W, b):
    import os
    from concourse.bass_utils import run_bass_kernel_spmd

    per_core, banks, iota, groups, totblk = _host_prep(
        x, edge_index, edge_weight)

    nc = _build_program(groups, totblk)

    WT = np.ascontiguousarray(np.asarray(W, np.float32).T)  # [din, dout]
    bcol = np.ascontiguousarray(np.asarray(b, np.float32).reshape(D, 1))

    in_maps = []
    for c in range(NCORES):
        p = per_core[c]
        m = {f"xb{i}": banks[i] for i in range(NBANKS)}
        m.update({
            "idx16": p["idx16"], "dl": p["dl"], "w": p["w"],
            "iota": iota, "wt": WT, "b": bcol,
        })
        in_maps.append(m)

    res = run_bass_kernel_spmd(
        nc, in_maps, core_ids=list(range(NCORES)),
        trace=bool(int(os.environ.get("KERNEL_TRACE", "0"))),
    )
    global LAST_RES
    LAST_RES = res

    out = np.empty((N_NODES, D), np.float32)
    for c in range(NCORES):
        outT = res.results[c]["outT"]  # [D, T*TILE_W]
        out[c * NLOC:(c + 1) * NLOC] = outT[:, :NLOC].T
    return out


if __name__ == "__main__":
    # smoke test with random data (self-contained)
    rng = np.random.default_rng(0)
    x = rng.standard_normal((N_NODES, D)).astype(np.float32)
    ei = rng.integers(0, N_NODES, size=(2, N_EDGES)).astype(np.int64)
    ew = rng.random(N_EDGES).astype(np.float32)
    W = (rng.standard_normal((D, D)) / np.sqrt(D)).astype(np.float32)
    b = (rng.standard_normal(D) * 0.01).astype(np.float32)
    out = kernel(x, ei, ew, W, b)
    print("out", out.shape, out.dtype)
